# revision 8
# baseline (speedup 1.0000x reference)
"""Masked dot-product attention on 8 Trainium2 NeuronCores.

Full inputs: queries/keys/values [16, 2048, 128] f32, valid_lens [16] int.
Data-parallel over batch: 2 batches per core, no cross-core communication.

Per-core math (batch b, Q=K=2048, D=128):
  S^T[k, q] = sum_d K[k,d] * (Q[q,d] * keep[q])        (PE, fp16)
  E[k, q]   = exp(S^T[k, q] / sqrt(D))                 (ACT, fp16 out)
  P[q, d+1] = sum_k E[k, q] * [V | 1][k, d+1]          (PE, fp16)
  out[q, d] = P[q, d] / P[q, D]                        (DVE)

Mask semantics match the reference exactly: the reference sets whole rows
q >= valid_len to -1e6, and softmax of a constant row is uniform 1/K.
Here keep[q] = 0 zeroes those rows' scores, exp(0) = 1 gives the same
uniform weights; unmasked rows differ from exp(s - max)/sum only by fp
rounding since softmax is shift-invariant (scores are O(1), no overflow).
"""

import math
from contextlib import ExitStack

import numpy as np

import concourse.bacc as bacc
import concourse.bass as bass
import concourse.tile as tile
from concourse import mybir
from concourse.bass_utils import run_bass_kernel_spmd
from concourse.masks import make_identity

B, Q, K, D = 16, 2048, 2048, 128
NCORES = 8
BLOC = B // NCORES          # batches per core
P = 128                     # partitions
NQT = Q // P                # 16 q-tiles per batch
NKT = K // P                # 16 k-tiles per batch
QBLK = 512                  # q columns per S^T matmul (moving free dim)
NQB = Q // QBLK             # 4 q-blocks per batch
CHUNK = 2                   # k-tiles per PSUM tile (one ACT drain)
DCH = 4                     # input DMA chunks per tensor
SCALE = 1.0 / math.sqrt(D)

F32 = mybir.dt.float32
F32R = mybir.dt.float32r
F16 = mybir.dt.float16


def _build_program() -> bass.Bass:
    nc = bacc.Bacc(name="attn_dp")

    q_d = nc.dram_tensor("q", [BLOC, Q, D], F32, kind="ExternalInput")
    k_d = nc.dram_tensor("k", [BLOC, K, D], F32, kind="ExternalInput")
    v_d = nc.dram_tensor("v", [BLOC, K, D], F32, kind="ExternalInput")
    keep_d = nc.dram_tensor("keep", [BLOC, Q], F32, kind="ExternalInput")
    out_d = nc.dram_tensor("out", [BLOC, Q, D], F32, kind="ExternalOutput")

    with tile.TileContext(nc) as tc, ExitStack() as ctx:
        singles = ctx.enter_context(tc.tile_pool(name="singles", bufs=1))
        nat = ctx.enter_context(tc.tile_pool(name="nat", bufs=2))
        big = ctx.enter_context(tc.tile_pool(name="big", bufs=2))
        epool = ctx.enter_context(tc.tile_pool(name="epool", bufs=2))
        small = ctx.enter_context(tc.tile_pool(name="small", bufs=4))
        outp = ctx.enter_context(tc.tile_pool(name="outp", bufs=4))
        ps_s = ctx.enter_context(tc.tile_pool(name="ps_s", bufs=3, space="PSUM"))
        ps_pv = ctx.enter_context(tc.tile_pool(name="ps_pv", bufs=2, space="PSUM"))

        ident = singles.tile([P, P], F16)
        make_identity(nc, ident)

        for b in range(BLOC):
            # ---- stage K^T: [d, k] in SBUF, via PE transpose of natural tiles
            k_nat = nat.tile([P, NKT, D], F32, tag="k_nat")
            k_src = k_d[b].rearrange("(t p) d -> p t d", p=P)
            for c in range(DCH):
                sl = slice(c * (NKT // DCH), (c + 1) * (NKT // DCH))
                nc.sync.dma_start(out=k_nat[:, sl, :], in_=k_src[:, sl, :])
            k16 = nat.tile([P, NKT, D], F16, tag="k16")
            nc.vector.tensor_copy(k16, k_nat)
            kT = big.tile([P, K], F16, tag="kT")
            for kt in range(NKT):
                pst = ps_pv.tile([P, D + 1], F16, tag="acc")
                nc.tensor.transpose(pst[:, 0:P], k16[:, kt, :], ident)
                nc.vector.tensor_copy(kT[:, kt * P : (kt + 1) * P], pst[:, 0:P])

            # ---- stage Q^T with the row mask folded in:
            # qT[:, q] = Q[q, :] * keep[q]  (matmul against diag(keep))
            keep_sb = small.tile([P, NQT], F32, tag="keep")
            nc.sync.dma_start(
                out=keep_sb, in_=keep_d[b].rearrange("(t p) -> p t", p=P)
            )
            q_nat = nat.tile([P, NQT, D], F32, tag="q_nat")
            q_src = q_d[b].rearrange("(t p) d -> p t d", p=P)
            for c in range(DCH):
                sl = slice(c * (NQT // DCH), (c + 1) * (NQT // DCH))
                nc.sync.dma_start(out=q_nat[:, sl, :], in_=q_src[:, sl, :])
            qT = big.tile([P, Q], F16, tag="qT")
            for qt in range(NQT):
                qm = small.tile([P, P], F16, tag="qm")
                nc.vector.tensor_scalar_mul(
                    qm, in0=q_nat[:, qt, :], scalar1=keep_sb[:, qt : qt + 1]
                )
                pst = ps_pv.tile([P, D + 1], F16, tag="acc")
                nc.tensor.transpose(pst[:, 0:P], qm, ident)
                nc.vector.tensor_copy(qT[:, qt * P : (qt + 1) * P], pst[:, 0:P])

            # ---- stage [V | 1] in fp16: [k, d+1] per k-tile
            v_nat = nat.tile([P, NKT, D], F32, tag="v_nat")
            v_src = v_d[b].rearrange("(t p) d -> p t d", p=P)
            for c in range(DCH):
                sl = slice(c * (NKT // DCH), (c + 1) * (NKT // DCH))
                nc.sync.dma_start(out=v_nat[:, sl, :], in_=v_src[:, sl, :])
            vb = big.tile([P, NKT, D + 1], F16, tag="vb")
            nc.vector.tensor_copy(vb[:, :, 0:D], v_nat)
            nc.vector.memset(vb[:, :, D : D + 1], 1.0)

            # ---- main loop over q-blocks, software-pipelined:
            # PV matmuls of q-block i-1 are interleaved between the S^T
            # quads of q-block i so the PE keeps streaming while ACT
            # drains exp; the ACT time hides entirely behind PV work.
            def emit_pv(e_prev, qb_prev, ql):
                qt = qb_prev * (QBLK // P) + ql
                pv = ps_pv.tile([P, D + 1], F32, tag="acc")
                for kt in range(NKT):
                    nc.tensor.matmul(
                        pv,
                        lhsT=e_prev[:, kt, ql * P : (ql + 1) * P],
                        rhs=vb[:, kt, :],
                        start=(kt == 0),
                        stop=(kt == NKT - 1),
                    )
                recip = small.tile([P, 1], F32, tag="recip")
                nc.vector.reciprocal(recip, pv[:, D : D + 1])
                o_sb = outp.tile([P, D], F32, tag="o")
                nc.vector.tensor_scalar_mul(o_sb, in0=pv[:, 0:D], scalar1=recip)
                nc.sync.dma_start(
                    out=out_d[b, qt * P : (qt + 1) * P, :], in_=o_sb
                )

            prev = None  # (e_sb, qb) of the previous q-block
            for qb in range(NQB):
                q_sl = qT[:, qb * QBLK : (qb + 1) * QBLK]
                e_sb = epool.tile([P, NKT, QBLK], F16, tag="e")
                for ch in range(NKT // CHUNK):
                    ps = ps_s.tile([P, CHUNK, QBLK], F32, tag="ps")
                    for j in range(CHUNK):
                        kt = ch * CHUNK + j
                        nc.tensor.matmul(
                            ps[:, j, :],
                            lhsT=kT[:, kt * P : (kt + 1) * P],
                            rhs=q_sl,
                        )
                    nc.scalar.activation(
                        out=e_sb[:, ch * CHUNK : (ch + 1) * CHUNK, :],
                        in_=ps,
                        func=mybir.ActivationFunctionType.Exp,
                        scale=SCALE,
                    )
                    if prev is not None and ch % 2 == 1:
                        emit_pv(prev[0], prev[1], ch // 2)
                prev = (e_sb, qb)
            for ql in range(QBLK // P):
                emit_pv(prev[0], prev[1], ql)
    nc.compile()
    return nc


_NC = None


def _get_nc() -> bass.Bass:
    global _NC
    if _NC is None:
        _NC = _build_program()
    return _NC


def _shard_inputs(queries, keys, values, valid_lens):
    queries = np.ascontiguousarray(np.asarray(queries, dtype=np.float32))
    keys = np.ascontiguousarray(np.asarray(keys, dtype=np.float32))
    values = np.ascontiguousarray(np.asarray(values, dtype=np.float32))
    valid_lens = np.asarray(valid_lens).astype(np.int64)
    keep = (np.arange(Q, dtype=np.int64)[None, :] < valid_lens[:, None]).astype(
        np.float32
    )
    in_maps = []
    for c in range(NCORES):
        lo, hi = c * BLOC, (c + 1) * BLOC
        in_maps.append(
            {
                "q": np.ascontiguousarray(queries[lo:hi]),
                "k": np.ascontiguousarray(keys[lo:hi]),
                "v": np.ascontiguousarray(values[lo:hi]),
                "keep": np.ascontiguousarray(keep[lo:hi]),
            }
        )
    return in_maps


def _run(inputs: dict, trace: bool = False):
    nc = _get_nc()
    in_maps = _shard_inputs(**inputs)
    res = run_bass_kernel_spmd(
        nc, in_maps, core_ids=list(range(NCORES)), trace=trace
    )
    out = np.concatenate([r["out"] for r in res.results], axis=0)
    return out, res


def kernel(**inputs) -> np.ndarray:
    out, _ = _run(inputs, trace=False)
    return out


# revision 10
# speedup vs baseline: 1.0391x; 1.0391x over previous
"""Masked dot-product attention on 8 Trainium2 NeuronCores.

Full inputs: queries/keys/values [16, 2048, 128] f32, valid_lens [16] int.
Data-parallel over batch: 2 batches per core, no cross-core communication.

Per-core math (batch b, Q=K=2048, D=128):
  S^T[k, q] = sum_d K[k,d] * (Q[q,d] * keep[q])        (PE, fp16)
  E[k, q]   = exp(S^T[k, q] / sqrt(D))                 (ACT, fp16 out)
  P[q, d+1] = sum_k E[k, q] * [V | 1][k, d+1]          (PE, fp16)
  out[q, d] = P[q, d] / P[q, D]                        (DVE)

Mask semantics match the reference exactly: the reference sets whole rows
q >= valid_len to -1e6, and softmax of a constant row is uniform 1/K.
Here keep[q] = 0 zeroes those rows' scores, exp(0) = 1 gives the same
uniform weights; unmasked rows differ from exp(s - max)/sum only by fp
rounding since softmax is shift-invariant (scores are O(1), no overflow).
"""

import math
from contextlib import ExitStack

import numpy as np

import concourse.bacc as bacc
import concourse.bass as bass
import concourse.tile as tile
from concourse import mybir
from concourse.bass_utils import run_bass_kernel_spmd
from concourse.masks import make_identity

B, Q, K, D = 16, 2048, 2048, 128
NCORES = 8
BLOC = B // NCORES          # batches per core
P = 128                     # partitions
NQT = Q // P                # 16 q-tiles per batch
NKT = K // P                # 16 k-tiles per batch
QBLK = 512                  # q columns per S^T matmul (moving free dim)
NQB = Q // QBLK             # 4 q-blocks per batch
CHUNK = 2                   # k-tiles per PSUM tile (one ACT drain)
DCH = 4                     # input DMA chunks per tensor
SCALE = 1.0 / math.sqrt(D)

F32 = mybir.dt.float32
F32R = mybir.dt.float32r
F16 = mybir.dt.float16


def _build_program() -> bass.Bass:
    nc = bacc.Bacc(name="attn_dp")

    q_d = nc.dram_tensor("q", [BLOC, Q, D], F32, kind="ExternalInput")
    k_d = nc.dram_tensor("k", [BLOC, K, D], F32, kind="ExternalInput")
    v_d = nc.dram_tensor("v", [BLOC, K, D], F32, kind="ExternalInput")
    keep_d = nc.dram_tensor("keep", [BLOC, Q], F32, kind="ExternalInput")
    out_d = nc.dram_tensor("out", [BLOC, Q, D], F32, kind="ExternalOutput")

    with tile.TileContext(nc) as tc, ExitStack() as ctx:
        singles = ctx.enter_context(tc.tile_pool(name="singles", bufs=1))
        nat = ctx.enter_context(tc.tile_pool(name="nat", bufs=2))
        big = ctx.enter_context(tc.tile_pool(name="big", bufs=2))
        epool = ctx.enter_context(tc.tile_pool(name="epool", bufs=2))
        small = ctx.enter_context(tc.tile_pool(name="small", bufs=4))
        outp = ctx.enter_context(tc.tile_pool(name="outp", bufs=4))
        ps_s = ctx.enter_context(tc.tile_pool(name="ps_s", bufs=3, space="PSUM"))
        ps_pv = ctx.enter_context(tc.tile_pool(name="ps_pv", bufs=2, space="PSUM"))

        ident = singles.tile([P, P], F16)
        make_identity(nc, ident)

        for b in range(BLOC):
            # ---- stage K^T: [d, k] in SBUF, via PE transpose of natural tiles
            k_nat = nat.tile([P, NKT, D], F32, tag="k_nat")
            k_src = k_d[b].rearrange("(t p) d -> p t d", p=P)
            for c in range(DCH):
                sl = slice(c * (NKT // DCH), (c + 1) * (NKT // DCH))
                nc.sync.dma_start(out=k_nat[:, sl, :], in_=k_src[:, sl, :])
            k16 = nat.tile([P, NKT, D], F16, tag="k16")
            for c in range(DCH):
                sl = slice(c * (NKT // DCH), (c + 1) * (NKT // DCH))
                nc.vector.tensor_copy(k16[:, sl, :], k_nat[:, sl, :])
            kT = big.tile([P, K], F16, tag="kT")
            for kt in range(NKT):
                pst = ps_pv.tile([P, D + 1], F16, tag="acc")
                nc.tensor.transpose(pst[:, 0:P], k16[:, kt, :], ident)
                nc.vector.tensor_copy(kT[:, kt * P : (kt + 1) * P], pst[:, 0:P])

            # ---- stage Q^T with the row mask folded in:
            # qT[:, q] = Q[q, :] * keep[q]  (matmul against diag(keep))
            keep_sb = small.tile([P, NQT], F32, tag="keep")
            nc.sync.dma_start(
                out=keep_sb, in_=keep_d[b].rearrange("(t p) -> p t", p=P)
            )
            q_nat = nat.tile([P, NQT, D], F32, tag="q_nat")
            q_src = q_d[b].rearrange("(t p) d -> p t d", p=P)
            for c in range(DCH):
                sl = slice(c * (NQT // DCH), (c + 1) * (NQT // DCH))
                nc.sync.dma_start(out=q_nat[:, sl, :], in_=q_src[:, sl, :])
            qT = big.tile([P, Q], F16, tag="qT")
            for qt in range(NQT):
                qm = small.tile([P, P], F16, tag="qm")
                nc.vector.tensor_scalar_mul(
                    qm, in0=q_nat[:, qt, :], scalar1=keep_sb[:, qt : qt + 1]
                )
                pst = ps_pv.tile([P, D + 1], F16, tag="acc")
                nc.tensor.transpose(pst[:, 0:P], qm, ident)
                nc.vector.tensor_copy(qT[:, qt * P : (qt + 1) * P], pst[:, 0:P])

            # ---- stage [V | 1] in fp16: [k, d+1] per k-tile
            v_nat = nat.tile([P, NKT, D], F32, tag="v_nat")
            v_src = v_d[b].rearrange("(t p) d -> p t d", p=P)
            for c in range(DCH):
                sl = slice(c * (NKT // DCH), (c + 1) * (NKT // DCH))
                nc.sync.dma_start(out=v_nat[:, sl, :], in_=v_src[:, sl, :])
            vb = big.tile([P, NKT, D + 1], F16, tag="vb")
            nc.vector.tensor_copy(vb[:, :, 0:D], v_nat)
            nc.vector.memset(vb[:, :, D : D + 1], 1.0)

            # ---- main loop over q-blocks, software-pipelined:
            # PV matmuls of q-block i-1 are interleaved between the S^T
            # quads of q-block i so the PE keeps streaming while ACT
            # drains exp; the ACT time hides entirely behind PV work.
            def emit_pv(e_prev, qb_prev, ql):
                qt = qb_prev * (QBLK // P) + ql
                pv = ps_pv.tile([P, D + 1], F32, tag="acc")
                for kt in range(NKT):
                    nc.tensor.matmul(
                        pv,
                        lhsT=e_prev[:, kt, ql * P : (ql + 1) * P],
                        rhs=vb[:, kt, :],
                        start=(kt == 0),
                        stop=(kt == NKT - 1),
                    )
                recip = small.tile([P, 1], F32, tag="recip")
                nc.vector.reciprocal(recip, pv[:, D : D + 1])
                o_sb = outp.tile([P, D], F32, tag="o")
                nc.vector.tensor_scalar_mul(o_sb, in0=pv[:, 0:D], scalar1=recip)
                nc.sync.dma_start(
                    out=out_d[b, qt * P : (qt + 1) * P, :], in_=o_sb
                )

            prev = None  # (e_sb, qb) of the previous q-block
            for qb in range(NQB):
                q_sl = qT[:, qb * QBLK : (qb + 1) * QBLK]
                e_sb = epool.tile([P, NKT, QBLK], F16, tag="e")
                for ch in range(NKT // CHUNK):
                    ps = ps_s.tile([P, CHUNK, QBLK], F32, tag="ps")
                    for j in range(CHUNK):
                        kt = ch * CHUNK + j
                        nc.tensor.matmul(
                            ps[:, j, :],
                            lhsT=kT[:, kt * P : (kt + 1) * P],
                            rhs=q_sl,
                        )
                    nc.scalar.activation(
                        out=e_sb[:, ch * CHUNK : (ch + 1) * CHUNK, :],
                        in_=ps,
                        func=mybir.ActivationFunctionType.Exp,
                        scale=SCALE,
                    )
                    if prev is not None and ch % 2 == 1:
                        emit_pv(prev[0], prev[1], ch // 2)
                prev = (e_sb, qb)
            for ql in range(QBLK // P):
                emit_pv(prev[0], prev[1], ql)
    nc.compile()
    return nc


_NC = None


def _get_nc() -> bass.Bass:
    global _NC
    if _NC is None:
        _NC = _build_program()
    return _NC


def _shard_inputs(queries, keys, values, valid_lens):
    queries = np.ascontiguousarray(np.asarray(queries, dtype=np.float32))
    keys = np.ascontiguousarray(np.asarray(keys, dtype=np.float32))
    values = np.ascontiguousarray(np.asarray(values, dtype=np.float32))
    valid_lens = np.asarray(valid_lens).astype(np.int64)
    keep = (np.arange(Q, dtype=np.int64)[None, :] < valid_lens[:, None]).astype(
        np.float32
    )
    in_maps = []
    for c in range(NCORES):
        lo, hi = c * BLOC, (c + 1) * BLOC
        in_maps.append(
            {
                "q": np.ascontiguousarray(queries[lo:hi]),
                "k": np.ascontiguousarray(keys[lo:hi]),
                "v": np.ascontiguousarray(values[lo:hi]),
                "keep": np.ascontiguousarray(keep[lo:hi]),
            }
        )
    return in_maps


def _run(inputs: dict, trace: bool = False):
    nc = _get_nc()
    in_maps = _shard_inputs(**inputs)
    res = run_bass_kernel_spmd(
        nc, in_maps, core_ids=list(range(NCORES)), trace=trace
    )
    out = np.concatenate([r["out"] for r in res.results], axis=0)
    return out, res


def kernel(**inputs) -> np.ndarray:
    out, _ = _run(inputs, trace=False)
    return out
